# revision 3
# baseline (speedup 1.0000x reference)
"""Trainium2 Bass kernel for nn_Meta_67078799229377 (relation-network meta-learner).

Sharding: 8 cores = 4 batch elements x 2 halves of the relation-j axis.
Each core runs the full backbone for its batch element's 6 images, then the
relation g-MLP for its 18 (i, j) pairs fully fused on-chip.  The device only
produces (a) per-image channel sums `fme` for the cls head and (b) the
(q,p)-summed relation features `xf`; the tiny f/cls MLP heads and loss
reductions run on the host in f64.

Engine plan (measured rates):
  - hdd = relu(v + u_q): DVE tensor_scalar at 4x-mode (~230ns / [128,384]),
    a configurable few on ACT.
  - g matmuls: PE pairs at tile_position (0,0)/(0,64) which overlap in the
    array (2 cols/cycle effective); 2048-col PSUM tiles, one per (unit, duo).
  - gscr relu+bias+sum: ACT activation with accum_out ([128,2048] ~2.15us).
  - Pool(gpsimd): memsets only (tensor ops are ~50x slow on HW).
  - PE warmup matmuls + early ACT table trigger overlap the input DMA.
"""
import os
import numpy as np
import ml_dtypes

import concourse.bass as bass
import concourse.mybir as mybir
import concourse.tile as tile
from concourse import bacc
from concourse.bass_utils import run_bass_kernel_spmd

F32 = mybir.dt.float32
BF16 = mybir.dt.bfloat16
AF = mybir.ActivationFunctionType
OP = mybir.AluOpType

B, S, D = 4, 6, 8
M = D * D            # 64 spatial positions
C2 = 66              # 64 channels + 2 coord channels
H1 = 128             # g-MLP hidden
CO = 64              # g-MLP out
NCls = 64
N_CORES = 8

# bf16 const blob column layout
CB_W1 = 0            # [27, 32]
CB_W2 = 32           # [32, 432]
CB_W3 = 464          # [48, 576]
CB_W1A = 1040        # [66, 128]
CB_W1B = 1168        # [66, 128]
CB_WG2 = 1296        # [128, 64]
CB_COORD = 1360      # [2, 384]
CB_N = 1744

# f32 const blob column layout: bc1, bc2, bc3, bg1, bg2(x2)
CF_N = 5

KWARM = int(os.environ.get("KWARM", "10"))
KH_ACT = int(os.environ.get("KH_ACT", "1"))   # hdd instrs per unit on ACT
KG_V = int(os.environ.get("KG_V", "0"))       # gscr instrs per unit on DVE


def _build_nc():
    nc = bacc.Bacc("TRN2", target_bir_lowering=False, debug=False,
                   num_devices=N_CORES)

    x_pk = nc.dram_tensor("pk", [128, 4, 512], BF16, kind="ExternalInput")
    x_cb = nc.dram_tensor("cb", [128, CB_N], BF16, kind="ExternalInput")
    x_cf = nc.dram_tensor("cf", [128, CF_N], F32, kind="ExternalInput")

    out_fme = nc.dram_tensor("fme", [64, S], F32, kind="ExternalOutput")
    out_xfa = nc.dram_tensor("xfa", [128, 18], F32, kind="ExternalOutput")
    out_xfd = nc.dram_tensor("xfd", [128, 18], F32, kind="ExternalOutput")

    with tile.TileContext(nc) as tc:
        with (
            tc.tile_pool(name="const", bufs=1) as cpool,
            tc.tile_pool(name="work", bufs=1) as wpool,
            tc.tile_pool(name="patch", bufs=1) as ppool,
            tc.tile_pool(name="hdd", bufs=2) as hpool,
            tc.tile_pool(name="gscr", bufs=2) as spool,
        ):
            # ---- scratch + warmup (no input deps: runs during DMA) ----
            wsrc = cpool.tile([128, 512], BF16, tag="wsrc")
            nc.gpsimd.memset(wsrc[:], 0.0)
            ttrig = wpool.tile([128, 2], F32, tag="ttrig")

            cb = cpool.tile([128, CB_N], BF16, tag="cb")
            cf = cpool.tile([128, CF_N], F32, tag="cf")
            nc.sync.dma_start(out=cb[:], in_=x_cb[:])
            nc.sync.dma_start(out=cf[:], in_=x_cf[:])

            patches_sb = ppool.tile([27, S, 1024], BF16)
            for c in range(3):
                nc.sync.dma_start(out=patches_sb[:, 2 * c:2 * c + 2, :],
                                  in_=x_patches[:, 2 * c:2 * c + 2, :])

            # trigger the ACT function-table load early (relu set)
            nc.scalar.activation(ttrig[:], wsrc[:, 0:2], AF.Relu)

            w1 = cb[0:27, CB_W1:CB_W1 + 32]
            w2 = cb[0:32, CB_W2:CB_W2 + 432]
            w3 = cb[0:48, CB_W3:CB_W3 + 576]
            w1a = cb[0:C2, CB_W1A:CB_W1A + H1]
            w1b = cb[0:C2, CB_W1B:CB_W1B + H1]
            wg2 = cb[:, CB_WG2:CB_WG2 + CO]
            bc1 = cf[0:32, 0:1]
            bc2 = cf[0:48, 1:2]
            bc3 = cf[0:64, 2:3]
            bg1 = cf[:, 3:4]
            bg2 = cf[:, 4:5]

            featc = wpool.tile([C2, S * M], BF16)
            nc.vector.tensor_copy(featc[64:66, :], cb[0:2, CB_COORD:CB_COORD + 384])

            xf_a = wpool.tile([128, 18], F32, tag="xfa")
            xf_d = wpool.tile([128, 18], F32, tag="xfd")
            nc.gpsimd.memset(xf_a[:], 0.0)
            nc.gpsimd.memset(xf_d[:], 0.0)

            zb2048 = cpool.tile([128, 2048], BF16, tag="zb")
            if KG_V > 0:
                nc.gpsimd.memset(zb2048[:], 0.0)

            c1sb = wpool.tile([32, S, 33, 33], BF16)
            c2sb = wpool.tile([48, S, 17, 17], BF16)
            for img in range(S):
                nc.gpsimd.memset(c1sb[:, img, 32, :], 0.0)
                nc.gpsimd.memset(c1sb[:, img, 0:32, 32], 0.0)
                nc.gpsimd.memset(c2sb[:, img, 16, :], 0.0)
                nc.gpsimd.memset(c2sb[:, img, 0:16, 16], 0.0)

            with tc.tile_pool(name="pwarm", bufs=1, space="PSUM") as pw_pool:
                psw = pw_pool.tile([128, 512], F32, tag="warm")
                for r in range(KWARM):
                    nc.tensor.matmul(psw[0:64, :], wsrc[:, 0:64], wsrc[:],
                                     start=True, stop=True, tile_position=(0, 0))
                    nc.tensor.matmul(psw[64:128, :], wsrc[:, 0:64], wsrc[:],
                                     start=True, stop=True, tile_position=(0, 64))

            with (
                tc.tile_pool(name="pconv", bufs=2, space="PSUM") as pc_pool,
                tc.tile_pool(name="psmall", bufs=2, space="PSUM") as ps_pool,
            ):
                # ---- conv1: [27]->[32], im2col'd, 64x64 -> 32x32 ----
                for img in range(S):
                    for h in range(2):
                        ps1 = pc_pool.tile([32, 16, 32], F32, tag="psc")
                        nc.tensor.matmul(
                            ps1[:].rearrange("p a b -> p (a b)"),
                            w1,
                            patches_sb[:, img, h * 512:(h + 1) * 512],
                            start=True, stop=True)
                        out_ap = c1sb[:, img, h * 16:(h + 1) * 16, 0:32]
                        if (img * 2 + h) % 2 == 0:
                            nc.scalar.activation(out_ap, ps1[:], AF.Relu, bias=bc1)
                        else:
                            nc.vector.tensor_scalar(out_ap, ps1[:], bc1, 0.0,
                                                    op0=OP.add, op1=OP.max)

                # ---- conv2: [32]->[48], 32x32 -> 16x16 ----
                for ip in range(3):
                    ps2 = pc_pool.tile([48, 2, 16, 16], F32, tag="psc")
                    for k, (dy, dx) in enumerate(
                            (dy, dx) for dy in range(3) for dx in range(3)):
                        nc.tensor.matmul(
                            ps2[:],
                            w2[:, k * 48:(k + 1) * 48],
                            c1sb[:, 2 * ip:2 * ip + 2, dy:dy + 31:2, dx:dx + 31:2],
                            start=(k == 0), stop=(k == 8))
                    out_ap = c2sb[:, 2 * ip:2 * ip + 2, 0:16, 0:16]
                    if ip % 2 == 0:
                        nc.scalar.activation(out_ap, ps2[:], AF.Relu, bias=bc2)
                    else:
                        nc.vector.tensor_scalar(out_ap, ps2[:], bc2, 0.0,
                                                op0=OP.add, op1=OP.max)

                # ---- conv3: [48]->[64], 16x16 -> 8x8 ----
                ps3 = ps_pool.tile([64, S, D, D], F32, tag="sm")
                for k, (dy, dx) in enumerate(
                        (dy, dx) for dy in range(3) for dx in range(3)):
                    nc.tensor.matmul(
                        ps3[:],
                        w3[:, k * 64:(k + 1) * 64],
                        c2sb[:, :, dy:dy + 15:2, dx:dx + 15:2],
                        start=(k == 0), stop=(k == 8))
                nc.scalar.activation(
                    featc[0:64, :].rearrange("p (i m) -> p i m", m=M),
                    ps3[:].rearrange("p i a b -> p i (a b)"),
                    AF.Relu, bias=bc3)

                # ---- cls: per-image channel sums, rest on host ----
                fme = wpool.tile([64, S], F32)
                nc.vector.tensor_reduce(
                    fme[:], featc[0:64, :].rearrange("p (i m) -> p i m", m=M),
                    axis=mybir.AxisListType.X, op=OP.add)
                nc.sync.dma_start(out=out_fme[:], in_=fme[:])

                # ---- u / v ----
                psu = ps_pool.tile([H1, S * M], F32, tag="sm")
                psv = ps_pool.tile([H1, S * M], F32, tag="sm")
                nc.tensor.matmul(psu[:], w1a, featc[:], start=True, stop=True)
                nc.tensor.matmul(psv[:], w1b, featc[:], start=True, stop=True)
                u_f32 = wpool.tile([H1, S * M], F32)
                v_bf = wpool.tile([H1, S * M], BF16)
                nc.scalar.activation(u_f32[:], psu[:], AF.Copy)
                nc.vector.tensor_scalar(v_bf[:], psv[:], bg1, None, op0=OP.add)

            # ---- relation stage ----
            hdd_act = set(range(32 - KH_ACT, 32))
            with tc.tile_pool(name="pbig", bufs=2, space="PSUM") as pb_pool:
                for jl in range(3):
                    for qh in range(2):
                        unit = jl * 2 + qh
                        hdd = hpool.tile([H1, 32, S * M], BF16, tag="hdd")
                        for ql in range(32):
                            q = qh * 32 + ql
                            ucol = u_f32[:, jl * M + q: jl * M + q + 1]
                            if ql in hdd_act:
                                nc.scalar.activation(hdd[:, ql, :], v_bf[:],
                                                     AF.Relu, bias=ucol)
                            else:
                                nc.vector.tensor_scalar(hdd[:, ql, :], v_bf[:],
                                                        ucol, 0.0,
                                                        op0=OP.add, op1=OP.max)
                        for duo in range(3):
                            iA, iB = 2 * duo, 2 * duo + 1
                            ps = pb_pool.tile([128, 2048], F32, tag="gps")
                            for qg in range(4):
                                nc.tensor.matmul(
                                    ps[0:CO, qg * 512:(qg + 1) * 512],
                                    wg2,
                                    hdd[:, qg * 8:(qg + 1) * 8, iA * M:(iA + 1) * M],
                                    start=True, stop=True,
                                    tile_position=(0, 0))
                                nc.tensor.matmul(
                                    ps[CO:2 * CO, qg * 512:(qg + 1) * 512],
                                    wg2,
                                    hdd[:, qg * 8:(qg + 1) * 8, iB * M:(iB + 1) * M],
                                    start=True, stop=True,
                                    tile_position=(0, 64))
                            col = unit * 3 + duo
                            gscr = spool.tile([128, 2048], BF16, tag="gscr")
                            if duo < KG_V:
                                nc.vector.scalar_tensor_tensor(
                                    gscr[:], ps[:], bg2, zb2048[:],
                                    op0=OP.add, op1=OP.max,
                                    accum_out=xf_d[:, col:col + 1])
                            else:
                                nc.scalar.activation(
                                    gscr[:], ps[:], AF.Relu, bias=bg2,
                                    accum_out=xf_a[:, col:col + 1])

            nc.sync.dma_start(out=out_xfa[:], in_=xf_a[:])
            nc.sync.dma_start(out=out_xfd[:], in_=xf_d[:])
    nc.compile()
    return nc


_NC_CACHE = None


def _get_nc():
    global _NC_CACHE
    if _NC_CACHE is None:
        _NC_CACHE = _build_nc()
    return _NC_CACHE


def _host_prep(inputs):
    ins = {k: np.asarray(v) for k, v in inputs.items()}
    x = np.concatenate([ins['support_x'], ins['query_x']], axis=1)
    lab = np.concatenate([ins['support_y'], ins['query_y']], axis=1)

    xpad = np.pad(x.astype(np.float32), ((0, 0), (0, 0), (0, 0), (0, 1), (0, 1)))
    win = np.lib.stride_tricks.sliding_window_view(xpad, (3, 3), axis=(3, 4))
    win = win[:, :, :, ::2, ::2]
    patches = win.transpose(0, 2, 5, 6, 1, 3, 4).reshape(B, 27, S, 1024)
    patches = np.ascontiguousarray(patches, np.float32)

    f32 = np.float32
    bf16 = ml_dtypes.bfloat16

    cb = np.zeros((128, CB_N), f32)
    cb[0:27, CB_W1:CB_W1 + 32] = ins['k1'].reshape(32, 27).T
    cb[0:32, CB_W2:CB_W2 + 432] = ins['k2'].transpose(1, 2, 3, 0).reshape(32, 432)
    cb[0:48, CB_W3:CB_W3 + 576] = ins['k3'].transpose(1, 2, 3, 0).reshape(48, 576)
    Wg1 = ins['Wg1'].astype(f32)
    cb[0:C2, CB_W1A:CB_W1A + H1] = Wg1[:C2]
    cb[0:C2, CB_W1B:CB_W1B + H1] = Wg1[C2:]
    cb[:, CB_WG2:CB_WG2 + CO] = ins['Wg2']
    ii = np.arange(D, dtype=f32) / D
    coord = np.stack([np.broadcast_to(ii[:, None], (D, D)),
                      np.broadcast_to(ii[None, :], (D, D))]).reshape(2, M)
    cb[0:2, CB_COORD:CB_COORD + 384] = np.tile(coord, (1, S))
    cb = cb.astype(bf16)

    cf = np.zeros((128, CF_N), f32)
    cf[0:32, 0] = ins['bc1']
    cf[0:48, 1] = ins['bc2']
    cf[0:64, 2] = ins['bc3']
    cf[:, 3] = ins['bg1']
    cf[:, 4] = np.tile(ins['bg2'].astype(f32), 2)

    in_maps = []
    for core in range(N_CORES):
        b, half = core // 2, core % 2
        perm = (0, 1, 2, 3, 4, 5) if half == 0 else (3, 4, 5, 0, 1, 2)
        m = dict(cb=cb, cf=cf)
        m['patches'] = np.ascontiguousarray(patches[b][:, perm, :]).astype(bf16)
        in_maps.append(m)
    return in_maps, lab, ins


def _host_post(results, lab, ins):
    f64 = np.float64
    Wf1 = ins['Wf1'].astype(f64)
    bf1 = ins['bf1'].astype(f64)
    Wf2 = ins['Wf2'].astype(f64)
    bf2 = ins['bf2'].astype(f64)
    Wlog = ins['Wlog'].astype(f64)
    blog = ins['blog'].astype(f64)

    P = np.zeros((B, S, S), f64)
    cls_terms = np.zeros((B, S), f64)
    for core in range(N_CORES):
        b, half = core // 2, core % 2
        perm = (0, 1, 2, 3, 4, 5) if half == 0 else (3, 4, 5, 0, 1, 2)
        xf = (results[core]["xfa"].astype(f64)
              + results[core]["xfd"].astype(f64))       # [128, 18]
        xf9 = xf.reshape(128, 3, 2, 3).sum(axis=2)      # (jl, duo)
        for jl in range(3):
            for duo in range(3):
                for par in range(2):
                    i = 2 * duo + par
                    vec = xf9[par * 64:(par + 1) * 64, jl, duo]
                    h = np.maximum(vec @ Wf1 + bf1, 0.0)
                    z = h @ Wf2 + bf2
                    P[b, perm[i], perm[jl]] = 1.0 / (1.0 + np.exp(-z[0]))
        if half == 0:
            fme = results[core]["fme"].astype(f64)      # [64, S] channel sums
            logits = (fme.T / M) @ Wlog + blog          # [S, NCls]
            mx = logits.max(axis=1)
            lse = mx + np.log(np.exp(logits - mx[:, None]).sum(axis=1))
            cls_terms[b] = lse - logits[np.arange(S), lab[b]]

    cls_loss = cls_terms.mean()
    y = (lab[:, :, None] == lab[:, None, :]).astype(f64)
    Pt = P.transpose(0, 2, 1)
    sym, anti = 0.5 * (P + Pt), 0.5 * (P - Pt)
    sym_n = np.sqrt((sym ** 2).sum(axis=(1, 2)))
    anti_n = np.sqrt((anti ** 2).sum(axis=(1, 2)))
    sym_loss = ((sym_n - anti_n) / (sym_n + anti_n)).mean()
    euc_loss = ((P - y) ** 2).mean()
    rn_loss = euc_loss - 0.1 * sym_loss
    return np.float32(cls_loss), np.float32(rn_loss), np.float32(sym_loss)


def run_spmd(inputs, trace=False, **kwargs):
    nc = _get_nc()
    in_maps, lab, ins = _host_prep(inputs)
    res = run_bass_kernel_spmd(nc, in_maps, list(range(N_CORES)),
                               trace=trace, **kwargs)
    return _host_post(res.results, lab, ins), res


def kernel(**inputs):
    out, _ = run_spmd(inputs)
    return out
